# revision 27
# baseline (speedup 1.0000x reference)
"""Multi-head attention Trainium2 Bass kernel.

Problem: B=8, S=1024, E=768, H=12, DH=64 MHA with per-head Q/K/V projections
and output projection. Data-parallel over batch: one batch element per
NeuronCore (8 cores).

Per-core dataflow (layouts keep the contraction dim on partitions; bf16
operands everywhere on the PE so LDWEIGHTS overlaps/FWL engages, fp32 PSUM
accumulation throughout):
  xT [E,S] bf16  <- DMA-transpose of x
  qT/kT/vT = W.T @ xT + b per head-pair [128(d-pair), S] bf16 (per-partition
     bias via DVE tensor_scalar_add on the PSUM->SBUF copy; weights are
     SBUF-resident, loaded once outside the rep loop)
  v [s, d-pair] <- PE transpose of vT, packed into vOnes with ones columns
     (softmax denominator comes free as row 64 of the attention matmul)
  scoresT[t,s] = k @ q.T per head, row-packed K=64 matmuls (two heads share
     the PE array via tile_position row strips)
  expST = exp(0.125*scoresT) (ACT, bf16; no max subtraction: scores ~ N(0,1))
  attnT[d,s| Z] = [v|1].T @ expST
  catT = attnT * (1/Z)  (DVE reciprocal + gpsimd partition_broadcast + TT)
  out = catT.T @ Wo (bf16 matmuls, fp32 out) + bo via TT against a
     partition-broadcast bias row (no K=1 bias matmul).
Output-projection PSUM tiles share the attention ring (tag "att") so the
QKV ring ("mm") stays free for the next rep's projections while the
outproj tail waits on the last pair's normalize.
"""
import sys

sys.path.insert(0, "/opt/trn_rl_repo")

import numpy as np
import ml_dtypes
from contextlib import ExitStack

import concourse.bass as bass
import concourse.tile as tile
from concourse import bacc, mybir
from concourse.bass_utils import run_bass_kernel_spmd
from concourse.masks import make_identity

F32 = mybir.dt.float32
BF16 = mybir.dt.bfloat16
AF = mybir.ActivationFunctionType
BF = ml_dtypes.bfloat16

B, S, E, H, DH = 8, 1024, 768, 12, 64
NP_ = 6          # head pairs
ET = 6           # e tiles of 128
ST = 8           # s tiles of 128
NCORES = 8

_cache = {}


def _build_nc(reps=1, ablate=""):
    if ("nc", reps, ablate) in _cache:
        return _cache[("nc", reps, ablate)]
    nc = bacc.Bacc("TRN2", target_bir_lowering=False, debug=False,
                   num_devices=NCORES)

    x = nc.dram_tensor("x", [S, E], BF16, kind="ExternalInput").ap()
    wq = nc.dram_tensor("wq", [NP_, 128, ET, 128], BF16, kind="ExternalInput").ap()
    wk = nc.dram_tensor("wk", [NP_, 128, ET, 128], BF16, kind="ExternalInput").ap()
    wv = nc.dram_tensor("wv", [NP_, 128, ET, 128], BF16, kind="ExternalInput").ap()
    bq = nc.dram_tensor("bq", [NP_, 128, 1], F32, kind="ExternalInput").ap()
    bk = nc.dram_tensor("bk", [NP_, 128, 1], F32, kind="ExternalInput").ap()
    bv = nc.dram_tensor("bv", [NP_, 128, 1], F32, kind="ExternalInput").ap()
    wo = nc.dram_tensor("wo", [128, ET * E], BF16, kind="ExternalInput").ap()
    bo = nc.dram_tensor("bo", [1, E], F32, kind="ExternalInput").ap()
    out = nc.dram_tensor("out", [S, E], F32, kind="ExternalOutput").ap()

    with tile.TileContext(nc) as tc, ExitStack() as ctx:
        consts = ctx.enter_context(tc.tile_pool(name="consts", bufs=1))
        xtp = ctx.enter_context(tc.tile_pool(
            name="xtp", bufs=2 if "db" in ablate else 1))
        catp = ctx.enter_context(tc.tile_pool(
            name="catp", bufs=2 if "db" in ablate else 1))
        qkp = ctx.enter_context(tc.tile_pool(
            name="qkp", bufs=2 if "qkp2" in ablate else 3))
        vop = ctx.enter_context(tc.tile_pool(
            name="vop", bufs=2 if "lag1" in ablate else 3))
        exq = ctx.enter_context(tc.tile_pool(
            name="exq", bufs=2 if "lag1" in ablate else 3))
        zp = ctx.enter_context(tc.tile_pool(name="zp", bufs=2))
        cup = ctx.enter_context(tc.tile_pool(
            name="cup", bufs=2 if "lag1" in ablate else 3))
        zdp = ctx.enter_context(tc.tile_pool(name="zdp", bufs=8, space="DRAM"))
        osb = ctx.enter_context(tc.tile_pool(name="osb", bufs=2))
        # PSUM: 2 + 4 + 2 = 8 banks (outproj shares the att ring)
        mmp = ctx.enter_context(tc.tile_pool(name="mmp", bufs=2, space="PSUM"))
        scp = ctx.enter_context(tc.tile_pool(
            name="scp", bufs=1 if "scp1" in ablate else 2, space="PSUM"))
        atp = ctx.enter_context(tc.tile_pool(name="atp", bufs=2, space="PSUM"))

        ident = consts.tile([128, 128], BF16, tag="ident")
        make_identity(nc, ident)
        wo_t = consts.tile([128, ET * E], BF16, tag="wo")
        nc.sync.dma_start(wo_t, wo)
        bo_rep = consts.tile([128, E], F32, tag="bo_rep")
        nc.sync.dma_start(bo_rep, bo.partition_broadcast(128))

        # SBUF-resident QKV weights+biases: loaded once, reused every rep.
        wq_t, wk_t, wv_t, bq_t, bk_t, bv_t = [], [], [], [], [], []
        for p in range(NP_):
            for src, dstl, wtag in ((wq, wq_t, "wq"), (wk, wk_t, "wk"),
                                    (wv, wv_t, "wv")):
                t = consts.tile([128, ET, 128], BF16, tag=f"{wtag}{p}",
                                name=f"{wtag}{p}")
                nc.sync.dma_start(t, src[p])
                dstl.append(t)
            for src, dstl, btag in ((bq, bq_t, "bq"), (bk, bk_t, "bk"),
                                    (bv, bv_t, "bv")):
                t = consts.tile([128, 1], F32, tag=f"{btag}{p}",
                                name=f"{btag}{p}")
                nc.sync.dma_start(t, src[p])
                dstl.append(t)

        for _rep in range(reps):
            # ---- Phase 0: xT [E, S] via DMA transpose ----
            xT = [xtp.tile([128, S], BF16, tag=f"xT{et}", name=f"xT{et}")
                  for et in range(ET)]
            for et in range(ET):
                nc.sync.dma_start(
                    xT[et], x[:, et * 128:(et + 1) * 128], transpose=True)

            catT = [catp.tile([128, S], BF16, tag=f"catT{j}", name=f"catT{j}")
                    for j in range(NP_)]

            # ---- Per head-pair, software-pipelined: pair p's attention
            # consumption (attn matmuls + normalize) is emitted after pair
            # p+1's production (QKV + scores + exp) so the PE always has
            # ready work while ACT runs the exps. ----
            def produce(p):
                qT = qkp.tile([128, S], BF16, tag="qT", name="qT")
                kT = qkp.tile([128, S], BF16, tag="kT", name="kT")
                vT = qkp.tile([128, S], BF16, tag="vT", name="vT", bufs=2)

                def proj(w_t, b_t, dst):
                    for ch in range(2):
                        pp = mmp.tile([128, 512], F32, tag="mm", name="pp")
                        for et in range(ET):
                            nc.tensor.matmul(
                                pp, w_t[:, et, :],
                                xT[et][:, ch * 512:(ch + 1) * 512],
                                start=(et == 0), stop=(et == ET - 1),
                            )
                        nc.vector.tensor_scalar_add(
                            dst[:, ch * 512:(ch + 1) * 512], pp, b_t)

                proj(wq_t[p], bq_t[p], qT)
                proj(wk_t[p], bk_t[p], kT)

                if ablate == "noattn":
                    for e in range(2):
                        nc.vector.memset(catT[p][64 * e:64 * e + 64, :], 0.5)
                    return None
                # scores for both heads interleaved (K=64 row-packing)
                ex_ts = [exq.tile([128, ST, S], BF16, tag=f"ex{e}",
                                  name=f"ex{e}") for e in range(2)]
                for t in range(ST):
                    scs = []
                    for e in range(2):
                        r0 = 64 * e
                        sc = scp.tile([128, S], F32, tag="sc", name="sc")
                        scs.append(sc)
                        for ch in range(2):
                            nc.tensor.matmul(
                                sc[:, ch * 512:(ch + 1) * 512],
                                kT[r0:r0 + 64, t * 128:(t + 1) * 128],
                                qT[r0:r0 + 64, ch * 512:(ch + 1) * 512],
                                tile_position=(r0, 0),
                                start=True, stop=True,
                                skip_group_check=True,
                            )
                    for e in range(2):
                        if ablate == "nosm":
                            continue
                        nc.scalar.activation(ex_ts[e][:, t, :], scs[e],
                                             AF.Exp, scale=0.125)

                # V projection + transpose emitted AFTER the scores so this
                # PE work fills the exp-gated window (vo isn't consumed
                # until two pairs later).
                proj(wv_t[p], bv_t[p], vT)
                vo = vop.tile([128, ST, 130], BF16, tag="vo", name="vo")
                nc.gpsimd.memset(vo.rearrange("p t d -> p (t d)"), 1.0)
                for sg in range(2):
                    tp = mmp.tile([128, 512], F32, tag="mm",
                                  name="tp").bitcast(BF16)
                    for k in range(4):
                        t = sg * 4 + k
                        nc.tensor.matmul(
                            tp[:, k * 128:(k + 1) * 128],
                            vT[:, t * 128:(t + 1) * 128],
                            ident, is_transpose=True, skip_group_check=True,
                            start=True, stop=True,
                        )
                    dst = vo[:, sg * 4:(sg + 1) * 4, :].rearrange(
                        "p t (two dd) -> p t two dd", two=2)[:, :, :, 0:64]
                    src = tp[:, 0:512].rearrange(
                        "p (t two d) -> p t two d", t=4, two=2)
                    nc.vector.tensor_copy(dst, src)

                if ablate in ("nosm", "noatmm"):
                    for e in range(2):
                        nc.vector.memset(catT[p][64 * e:64 * e + 64, :], 0.5)
                    return None
                return (vo, ex_ts)

            def consume(p, state):
                if state is None:
                    return
                vo, ex_ts = state
                # Stage unnormalized attention into catU and release each
                # attention PSUM tile quickly (recip + copy only); the
                # gpsimd broadcast and the normalizing TT run off the
                # critical path. ch-major order so outproj's first s-tiles
                # unblock after two TTs.
                catU = cup.tile([128, S], BF16, tag="catU", name="catU")
                deferred = []
                for ch in range(2):
                    for e in range(2):
                        r0 = 64 * e
                        ex_t = ex_ts[e]
                        ap_ = atp.tile([65, 512], F32, tag="att", name="att")
                        for t in range(ST):
                            nc.tensor.matmul(
                                ap_, vo[:, t, 65 * e:65 * e + 65],
                                ex_t[:, t, ch * 512:(ch + 1) * 512],
                                start=(t == 0), stop=(t == ST - 1),
                            )
                        zrec = zp.tile([1, 512], F32, tag="zrec", name="zrec",
                                       bufs=4)
                        from contextlib import nullcontext
                        prio = (nullcontext() if "lopri" in ablate
                                else tc.high_priority(offset=150))
                        with prio:
                            nc.vector.reciprocal(zrec, ap_[64:65, :])
                            nc.vector.tensor_copy(
                                catU[r0:r0 + 64, ch * 512:(ch + 1) * 512],
                                ap_[0:64, :])
                        zd = zdp.tile([1, 512], F32, tag="zd", name="zd")
                        nc.sync.dma_start(zd, zrec)
                        zrep = zp.tile([128, 512], F32, tag="zrep",
                                       name="zrep", bufs=4)
                        nc.sync.dma_start(
                            zrep, zd.partition_broadcast(128))
                        deferred.append((r0, ch, zrep))
                tt_eng = nc.vector if "ttdve" in ablate else nc.gpsimd
                for r0, ch, zrep in deferred:
                    tt_eng.tensor_tensor(
                        out=catT[p][r0:r0 + 64, ch * 512:(ch + 1) * 512],
                        in0=catU[r0:r0 + 64, ch * 512:(ch + 1) * 512],
                        in1=zrep[r0:r0 + 64, :],
                        op=mybir.AluOpType.mult,
                    )

            if "lag1" not in ablate:
                states = {}
                for p in range(NP_):
                    states[p] = produce(p)
                    if p >= 2:
                        consume(p - 2, states.pop(p - 2))
                consume(NP_ - 2, states.pop(NP_ - 2))
                consume(NP_ - 1, states.pop(NP_ - 1))
            else:
                prev = None
                for p in range(NP_):
                    state = produce(p)
                    if p >= 1:
                        consume(p - 1, prev)
                    prev = state
                consume(NP_ - 1, prev)

            # ---- Output projection (PSUM from the att ring; bias via TT
            # against the broadcast bo row, off the PE) ----
            for st in range(ST):
                for ch in range(2):
                    op_ = atp.tile([128, 384], F32, tag="att", name="op")
                    for j in range(NP_):
                        nc.tensor.matmul(
                            op_, catT[j][:, st * 128:(st + 1) * 128],
                            wo_t[:, j * E + ch * 384:j * E + ch * 384 + 384],
                            start=(j == 0), stop=(j == NP_ - 1),
                        )
                    o_sb = osb.tile([128, 384], F32, tag="ot", name="ot")
                    nc.vector.tensor_tensor(
                        out=o_sb, in0=op_,
                        in1=bo_rep[:, ch * 384:ch * 384 + 384],
                        op=mybir.AluOpType.add,
                    )
                    nc.sync.dma_start(
                        out[st * 128:(st + 1) * 128,
                            ch * 384:ch * 384 + 384], o_sb)

    nc.compile()
    _cache[("nc", reps, ablate)] = nc
    return nc


def _prep_weights(Wq, bq, Wk, bk, Wv, bv, Wo, bo):
    def pack_w(W):  # [12, 768, 64] -> [6, 128, 6, 128] bf16
        Wp = W.reshape(NP_, 2, E, DH).transpose(0, 2, 1, 3).reshape(NP_, E, 128)
        return np.ascontiguousarray(
            Wp.reshape(NP_, ET, 128, 128).transpose(0, 2, 1, 3)).astype(BF)

    def pack_b(b):  # [12, 64] -> [6, 128, 1] f32
        return np.ascontiguousarray(b.reshape(NP_, 128, 1)).astype(np.float32)

    return {
        "wq": pack_w(Wq), "wk": pack_w(Wk), "wv": pack_w(Wv),
        "bq": pack_b(bq), "bk": pack_b(bk), "bv": pack_b(bv),
        "wo": np.ascontiguousarray(
            Wo.reshape(ET, 128, E).transpose(1, 0, 2).reshape(128, ET * E)
        ).astype(BF),
        "bo": np.ascontiguousarray(bo.reshape(1, E)).astype(np.float32),
    }


def kernel(hidden_state, Wq, bq, Wk, bk, Wv, bv, Wo, bo):
    hidden_state = np.asarray(hidden_state, dtype=np.float32)
    shared = _prep_weights(
        np.asarray(Wq, np.float32), np.asarray(bq, np.float32),
        np.asarray(Wk, np.float32), np.asarray(bk, np.float32),
        np.asarray(Wv, np.float32), np.asarray(bv, np.float32),
        np.asarray(Wo, np.float32), np.asarray(bo, np.float32))
    nc = _build_nc()
    in_maps = [
        {"x": np.ascontiguousarray(hidden_state[b]).astype(BF), **shared}
        for b in range(NCORES)
    ]
    res = run_bass_kernel_spmd(nc, in_maps, core_ids=list(range(NCORES)))
    return np.stack([r["out"] for r in res.results], axis=0)


# revision 28
# speedup vs baseline: 1.0766x; 1.0766x over previous
"""Multi-head attention Trainium2 Bass kernel.

Problem: B=8, S=1024, E=768, H=12, DH=64 MHA with per-head Q/K/V projections
and output projection. Data-parallel over batch: one batch element per
NeuronCore (8 cores).

Per-core dataflow (layouts keep the contraction dim on partitions; bf16
operands everywhere on the PE so LDWEIGHTS overlaps/FWL engages, fp32 PSUM
accumulation throughout):
  xT [E,S] bf16  <- DMA-transpose of x
  qT/kT/vT = W.T @ xT + b per head-pair [128(d-pair), S] bf16 (per-partition
     bias via DVE tensor_scalar_add on the PSUM->SBUF copy; weights are
     SBUF-resident, loaded once outside the rep loop)
  v [s, d-pair] <- PE transpose of vT, packed into vOnes with ones columns
     (softmax denominator comes free as row 64 of the attention matmul)
  scoresT[t,s] = k @ q.T per head, row-packed K=64 matmuls (two heads share
     the PE array via tile_position row strips)
  expST = exp(0.125*scoresT) (ACT, bf16; no max subtraction: scores ~ N(0,1))
  attnT[d,s| Z] = [v|1].T @ expST
  catT = attnT * (1/Z)  (DVE reciprocal + gpsimd partition_broadcast + TT)
  out = catT.T @ Wo (bf16 matmuls, fp32 out) + bo via TT against a
     partition-broadcast bias row (no K=1 bias matmul).
Output-projection PSUM tiles share the attention ring (tag "att") so the
QKV ring ("mm") stays free for the next rep's projections while the
outproj tail waits on the last pair's normalize.
"""
import sys

sys.path.insert(0, "/opt/trn_rl_repo")

import numpy as np
import ml_dtypes
from contextlib import ExitStack

import concourse.bass as bass
import concourse.tile as tile
from concourse import bacc, mybir
from concourse.bass_utils import run_bass_kernel_spmd
from concourse.masks import make_identity

F32 = mybir.dt.float32
BF16 = mybir.dt.bfloat16
AF = mybir.ActivationFunctionType
BF = ml_dtypes.bfloat16

B, S, E, H, DH = 8, 1024, 768, 12, 64
NP_ = 6          # head pairs
ET = 6           # e tiles of 128
ST = 8           # s tiles of 128
NCORES = 8

_cache = {}


def _build_nc(reps=1, ablate=""):
    if ("nc", reps, ablate) in _cache:
        return _cache[("nc", reps, ablate)]
    nc = bacc.Bacc("TRN2", target_bir_lowering=False, debug=False,
                   num_devices=NCORES)

    x = nc.dram_tensor("x", [S, E], BF16, kind="ExternalInput").ap()
    wq = nc.dram_tensor("wq", [NP_, 128, ET, 128], BF16, kind="ExternalInput").ap()
    wk = nc.dram_tensor("wk", [NP_, 128, ET, 128], BF16, kind="ExternalInput").ap()
    wv = nc.dram_tensor("wv", [NP_, 128, ET, 128], BF16, kind="ExternalInput").ap()
    bq = nc.dram_tensor("bq", [NP_, 128, 1], F32, kind="ExternalInput").ap()
    bk = nc.dram_tensor("bk", [NP_, 128, 1], F32, kind="ExternalInput").ap()
    bv = nc.dram_tensor("bv", [NP_, 128, 1], F32, kind="ExternalInput").ap()
    wo = nc.dram_tensor("wo", [128, ET * E], BF16, kind="ExternalInput").ap()
    bo = nc.dram_tensor("bo", [1, E], F32, kind="ExternalInput").ap()
    out = nc.dram_tensor("out", [S, E], F32, kind="ExternalOutput").ap()

    with tile.TileContext(nc) as tc, ExitStack() as ctx:
        consts = ctx.enter_context(tc.tile_pool(name="consts", bufs=1))
        xtp = ctx.enter_context(tc.tile_pool(
            name="xtp", bufs=2 if "db" in ablate else 1))
        catp = ctx.enter_context(tc.tile_pool(
            name="catp", bufs=2 if "db" in ablate else 1))
        qkp = ctx.enter_context(tc.tile_pool(
            name="qkp", bufs=2 if "qkp2" in ablate else 3))
        vop = ctx.enter_context(tc.tile_pool(
            name="vop", bufs=2 if "lag1" in ablate else 3))
        exq = ctx.enter_context(tc.tile_pool(
            name="exq", bufs=2 if "lag1" in ablate else 3))
        zp = ctx.enter_context(tc.tile_pool(name="zp", bufs=2))
        cup = ctx.enter_context(tc.tile_pool(
            name="cup", bufs=2 if "lag1" in ablate else 3))
        zdp = ctx.enter_context(tc.tile_pool(name="zdp", bufs=8, space="DRAM"))
        osb = ctx.enter_context(tc.tile_pool(name="osb", bufs=2))
        # PSUM: 2 + 4 + 2 = 8 banks (outproj shares the att ring)
        mmp = ctx.enter_context(tc.tile_pool(name="mmp", bufs=2, space="PSUM"))
        scp = ctx.enter_context(tc.tile_pool(
            name="scp", bufs=1 if "scp1" in ablate else 2, space="PSUM"))
        atp = ctx.enter_context(tc.tile_pool(name="atp", bufs=2, space="PSUM"))

        ident = consts.tile([128, 128], BF16, tag="ident")
        make_identity(nc, ident)
        wo_t = consts.tile([128, ET * E], BF16, tag="wo")
        nc.sync.dma_start(wo_t, wo)
        bo_rep = consts.tile([128, E], F32, tag="bo_rep")
        nc.sync.dma_start(bo_rep, bo.partition_broadcast(128))

        # SBUF-resident QKV weights+biases: loaded once, reused every rep.
        wq_t, wk_t, wv_t, bq_t, bk_t, bv_t = [], [], [], [], [], []
        for p in range(NP_):
            for src, dstl, wtag in ((wq, wq_t, "wq"), (wk, wk_t, "wk"),
                                    (wv, wv_t, "wv")):
                t = consts.tile([128, ET, 128], BF16, tag=f"{wtag}{p}",
                                name=f"{wtag}{p}")
                nc.sync.dma_start(t, src[p])
                dstl.append(t)
            for src, dstl, btag in ((bq, bq_t, "bq"), (bk, bk_t, "bk"),
                                    (bv, bv_t, "bv")):
                t = consts.tile([128, 1], F32, tag=f"{btag}{p}",
                                name=f"{btag}{p}")
                nc.sync.dma_start(t, src[p])
                dstl.append(t)

        for _rep in range(reps):
            # ---- Phase 0: xT [E, S] via DMA transpose ----
            xT = [xtp.tile([128, S], BF16, tag=f"xT{et}", name=f"xT{et}")
                  for et in range(ET)]
            for et in range(ET):
                nc.sync.dma_start(
                    xT[et], x[:, et * 128:(et + 1) * 128], transpose=True)

            catT = [catp.tile([128, S], BF16, tag=f"catT{j}", name=f"catT{j}")
                    for j in range(NP_)]

            # ---- Per head-pair, software-pipelined: pair p's attention
            # consumption (attn matmuls + normalize) is emitted after pair
            # p+1's production (QKV + scores + exp) so the PE always has
            # ready work while ACT runs the exps. ----
            def produce(p):
                qT = qkp.tile([128, S], BF16, tag="qT", name="qT")
                kT = qkp.tile([128, S], BF16, tag="kT", name="kT")
                vT = qkp.tile([128, S], BF16, tag="vT", name="vT", bufs=2)
                for w_t, b_t, dst in ((wq_t[p], bq_t[p], qT),
                                      (wk_t[p], bk_t[p], kT),
                                      (wv_t[p], bv_t[p], vT)):
                    for ch in range(2):
                        pp = mmp.tile([128, 512], F32, tag="mm", name="pp")
                        for et in range(ET):
                            nc.tensor.matmul(
                                pp, w_t[:, et, :],
                                xT[et][:, ch * 512:(ch + 1) * 512],
                                start=(et == 0), stop=(et == ET - 1),
                            )
                        nc.vector.tensor_scalar_add(
                            dst[:, ch * 512:(ch + 1) * 512], pp, b_t)

                # transpose vT -> v [s, d-pair] packed into vOnes w/ ones cols
                vo = vop.tile([128, ST, 130], BF16, tag="vo", name="vo")
                nc.gpsimd.memset(vo.rearrange("p t d -> p (t d)"), 1.0)
                for sg in range(2):
                    tp = mmp.tile([128, 512], F32, tag="mm",
                                  name="tp").bitcast(BF16)
                    for k in range(4):
                        t = sg * 4 + k
                        nc.tensor.matmul(
                            tp[:, k * 128:(k + 1) * 128],
                            vT[:, t * 128:(t + 1) * 128],
                            ident, is_transpose=True, skip_group_check=True,
                            start=True, stop=True,
                        )
                    dst = vo[:, sg * 4:(sg + 1) * 4, :].rearrange(
                        "p t (two dd) -> p t two dd", two=2)[:, :, :, 0:64]
                    src = tp[:, 0:512].rearrange(
                        "p (t two d) -> p t two d", t=4, two=2)
                    nc.vector.tensor_copy(dst, src)

                if ablate == "noattn":
                    for e in range(2):
                        nc.vector.memset(catT[p][64 * e:64 * e + 64, :], 0.5)
                    return None
                # scores for both heads interleaved (K=64 row-packing)
                ex_ts = [exq.tile([128, ST, S], BF16, tag=f"ex{e}",
                                  name=f"ex{e}") for e in range(2)]
                for t in range(ST):
                    scs = []
                    for e in range(2):
                        r0 = 64 * e
                        sc = scp.tile([128, S], F32, tag="sc", name="sc")
                        scs.append(sc)
                        for ch in range(2):
                            nc.tensor.matmul(
                                sc[:, ch * 512:(ch + 1) * 512],
                                kT[r0:r0 + 64, t * 128:(t + 1) * 128],
                                qT[r0:r0 + 64, ch * 512:(ch + 1) * 512],
                                tile_position=(r0, 0),
                                start=True, stop=True,
                                skip_group_check=True,
                            )
                    for e in range(2):
                        if ablate == "nosm":
                            continue
                        nc.scalar.activation(ex_ts[e][:, t, :], scs[e],
                                             AF.Exp, scale=0.125)
                if ablate in ("nosm", "noatmm"):
                    for e in range(2):
                        nc.vector.memset(catT[p][64 * e:64 * e + 64, :], 0.5)
                    return None
                return (vo, ex_ts)

            def consume(p, state):
                if state is None:
                    return
                vo, ex_ts = state
                # Stage unnormalized attention into catU and release each
                # attention PSUM tile quickly (recip + copy only); the
                # gpsimd broadcast and the normalizing TT run off the
                # critical path. ch-major order so outproj's first s-tiles
                # unblock after two TTs.
                catU = cup.tile([128, S], BF16, tag="catU", name="catU")
                deferred = []
                for ch in range(2):
                    for e in range(2):
                        r0 = 64 * e
                        ex_t = ex_ts[e]
                        ap_ = atp.tile([65, 512], F32, tag="att", name="att")
                        for t in range(ST):
                            nc.tensor.matmul(
                                ap_, vo[:, t, 65 * e:65 * e + 65],
                                ex_t[:, t, ch * 512:(ch + 1) * 512],
                                start=(t == 0), stop=(t == ST - 1),
                            )
                        zrec = zp.tile([1, 512], F32, tag="zrec", name="zrec",
                                       bufs=4)
                        from contextlib import nullcontext
                        prio = (nullcontext() if "lopri" in ablate
                                else tc.high_priority(offset=150))
                        with prio:
                            nc.vector.reciprocal(zrec, ap_[64:65, :])
                            nc.vector.tensor_copy(
                                catU[r0:r0 + 64, ch * 512:(ch + 1) * 512],
                                ap_[0:64, :])
                        zd = zdp.tile([1, 512], F32, tag="zd", name="zd")
                        nc.sync.dma_start(zd, zrec)
                        zrep = zp.tile([128, 512], F32, tag="zrep",
                                       name="zrep", bufs=4)
                        nc.sync.dma_start(
                            zrep, zd.partition_broadcast(128))
                        deferred.append((r0, ch, zrep))
                tt_eng = nc.vector if "ttdve" in ablate else nc.gpsimd
                for r0, ch, zrep in deferred:
                    tt_eng.tensor_tensor(
                        out=catT[p][r0:r0 + 64, ch * 512:(ch + 1) * 512],
                        in0=catU[r0:r0 + 64, ch * 512:(ch + 1) * 512],
                        in1=zrep[r0:r0 + 64, :],
                        op=mybir.AluOpType.mult,
                    )

            if "lag1" not in ablate:
                states = {}
                for p in range(NP_):
                    states[p] = produce(p)
                    if p >= 2:
                        consume(p - 2, states.pop(p - 2))
                consume(NP_ - 2, states.pop(NP_ - 2))
                consume(NP_ - 1, states.pop(NP_ - 1))
            else:
                prev = None
                for p in range(NP_):
                    state = produce(p)
                    if p >= 1:
                        consume(p - 1, prev)
                    prev = state
                consume(NP_ - 1, prev)

            # ---- Output projection (PSUM from the att ring; bias via TT
            # against the broadcast bo row, off the PE) ----
            for st in range(ST):
                for ch in range(2):
                    op_ = atp.tile([128, 384], F32, tag="att", name="op")
                    for j in range(NP_):
                        nc.tensor.matmul(
                            op_, catT[j][:, st * 128:(st + 1) * 128],
                            wo_t[:, j * E + ch * 384:j * E + ch * 384 + 384],
                            start=(j == 0), stop=(j == NP_ - 1),
                        )
                    o_sb = osb.tile([128, 384], F32, tag="ot", name="ot")
                    nc.vector.tensor_tensor(
                        out=o_sb, in0=op_,
                        in1=bo_rep[:, ch * 384:ch * 384 + 384],
                        op=mybir.AluOpType.add,
                    )
                    nc.sync.dma_start(
                        out[st * 128:(st + 1) * 128,
                            ch * 384:ch * 384 + 384], o_sb)

    nc.compile()
    _cache[("nc", reps, ablate)] = nc
    return nc


def _prep_weights(Wq, bq, Wk, bk, Wv, bv, Wo, bo):
    def pack_w(W):  # [12, 768, 64] -> [6, 128, 6, 128] bf16
        Wp = W.reshape(NP_, 2, E, DH).transpose(0, 2, 1, 3).reshape(NP_, E, 128)
        return np.ascontiguousarray(
            Wp.reshape(NP_, ET, 128, 128).transpose(0, 2, 1, 3)).astype(BF)

    def pack_b(b):  # [12, 64] -> [6, 128, 1] f32
        return np.ascontiguousarray(b.reshape(NP_, 128, 1)).astype(np.float32)

    return {
        "wq": pack_w(Wq), "wk": pack_w(Wk), "wv": pack_w(Wv),
        "bq": pack_b(bq), "bk": pack_b(bk), "bv": pack_b(bv),
        "wo": np.ascontiguousarray(
            Wo.reshape(ET, 128, E).transpose(1, 0, 2).reshape(128, ET * E)
        ).astype(BF),
        "bo": np.ascontiguousarray(bo.reshape(1, E)).astype(np.float32),
    }


def kernel(hidden_state, Wq, bq, Wk, bk, Wv, bv, Wo, bo):
    hidden_state = np.asarray(hidden_state, dtype=np.float32)
    shared = _prep_weights(
        np.asarray(Wq, np.float32), np.asarray(bq, np.float32),
        np.asarray(Wk, np.float32), np.asarray(bk, np.float32),
        np.asarray(Wv, np.float32), np.asarray(bv, np.float32),
        np.asarray(Wo, np.float32), np.asarray(bo, np.float32))
    nc = _build_nc()
    in_maps = [
        {"x": np.ascontiguousarray(hidden_state[b]).astype(BF), **shared}
        for b in range(NCORES)
    ]
    res = run_bass_kernel_spmd(nc, in_maps, core_ids=list(range(NCORES)))
    return np.stack([r["out"] for r in res.results], axis=0)
